# revision 8
# baseline (speedup 1.0000x reference)
"""Trainium2 Bass kernel for a single-layer decoder-only transformer.

Problem shapes: B=2, S=2048, D=1024, M=4096, OMEGA=32000 (fp32 reference).

Sharding: 8 cores; core c owns the 512-query chunk q0=(c%4)*512 of batch
b=c//4 (token-parallel).  Every core runs the identical SPMD program:

  A. gather emb rows for its batch's 2048 context tokens (+ its 512 query
     tokens), add positional encoding, transpose to feature-major h^T.
  B. K^T = Wk^T h^T, V = h^T^T Wv (token-major), Q^T = Wq^T hq^T.
  C. causal attention for the 512 queries over the full 2048-key context
     (host-provided masks make causality data-driven so the program is
     uniform across cores), then Wo projection and the gelu FFN, all kept
     feature-major: ffT [4096, 512].
  D. logits = ffT^T @ Wl + bl, streamed over the full 32000-col vocab;
     each core writes its own [512, 32000] fp32 output slab.

All matmuls run in bf16 (inputs rounded host-side for weights, on-device
for activations) with fp32 PSUM accumulation; measured end-to-end error
vs the fp32 reference is ~0.5% relative.
"""

import numpy as np
import ml_dtypes

import concourse.bass as bass
import concourse.bacc as bacc
import concourse.tile as tile
import concourse.mybir as mybir
from concourse.bass_utils import run_bass_kernel_spmd
from concourse.masks import make_identity

P = 128
B, S, D, M, V = 2, 2048, 1024, 4096, 32000
TQ = 512              # queries per core
CTX = S               # context length (uniform across cores)
NCORES = 8
DC = D // P           # 8 feature chunks
MC = M // P           # 32 ffn chunks
KC = CTX // P         # 16 key chunks
QS = TQ // P          # 4 query subtiles
N_TILE = 512
N_TILES = [(i * N_TILE, min(N_TILE, V - i * N_TILE)) for i in range((V + N_TILE - 1) // N_TILE)]

F32 = mybir.dt.float32
BF16 = mybir.dt.bfloat16
I32 = mybir.dt.int32
AF = mybir.ActivationFunctionType
AX = mybir.AxisListType

_CACHE = {}


def _bcast_ap(t, offset, n, length):
    """DRAM AP broadcasting a [length] row to [n, length] partitions."""
    return bass.AP(tensor=t.tensor, offset=offset, ap=[[0, n], [1, length]])


def build_program():
    nc = bacc.Bacc("TRN2", target_bir_lowering=False, debug=False,
                   num_devices=NCORES)

    def din(name, shape, dt):
        return nc.dram_tensor(name, shape, dt, kind="ExternalInput").ap()

    xk = din("xk", [CTX, 1], I32)
    xq = din("xq", [TQ, 1], I32)
    emb = din("emb", [V, D], F32)
    pek = din("pek", [CTX, D], F32)
    peq = din("peq", [TQ, D], F32)
    wq = din("wq", [D, D], BF16)
    wk = din("wk", [D, D], BF16)
    wv = din("wv", [D, D], BF16)
    wo = din("wo", [D, D], BF16)
    wf = din("wf", [D, M], BF16)
    wl = din("wl", [M, V], BF16)
    bq = din("bq", [D], F32)
    bk = din("bk", [D], F32)
    bv = din("bv", [D], F32)
    bo = din("bo", [D], F32)
    bf_ = din("bf", [M], F32)
    bl = din("bl", [V], F32)
    maskq = din("maskq", [QS, P, CTX], BF16)
    out = nc.dram_tensor("out", [TQ, V], F32, kind="ExternalOutput").ap()
    ktmp = nc.dram_tensor("ktmp", [D, CTX], BF16, kind="Internal").ap()
    ktmp_r = ktmp.rearrange("(c p) s -> p c s", p=P)

    wq_r = wq.rearrange("(c p) o -> p c o", p=P)
    wk_r = wk.rearrange("(c p) o -> p c o", p=P)
    wv_r = wv.rearrange("(c p) o -> p c o", p=P)
    wo_r = wo.rearrange("(c p) o -> p c o", p=P)
    wf_r = wf.rearrange("(c p) o -> p c o", p=P)
    wl_r = wl.rearrange("(c p) o -> p c o", p=P)

    with tile.TileContext(nc) as tc:
        _emit(nc, tc, locals())
    nc.compile()
    return nc


def _emit(nc, tc, t):
    import contextlib
    ctx = contextlib.ExitStack()
    with ctx:
        main = ctx.enter_context(tc.tile_pool(name="main", bufs=1))
        poolFF = ctx.enter_context(tc.tile_pool(name="poolFF", bufs=1))
        stC = ctx.enter_context(contextlib.ExitStack())
        poolC = stC.enter_context(tc.tile_pool(name="poolC", bufs=1))
        stW = stC.enter_context(contextlib.ExitStack())
        poolW = stW.enter_context(tc.tile_pool(name="poolW", bufs=1))
        stAB = stW.enter_context(contextlib.ExitStack())
        poolAB = stAB.enter_context(tc.tile_pool(name="poolAB", bufs=1))
        # PSUM pools (own LIFO stack)
        stPB = ctx.enter_context(contextlib.ExitStack())
        psB = stPB.enter_context(tc.tile_pool(name="psB", bufs=2, space="PSUM"))
        stPA = stPB.enter_context(contextlib.ExitStack())
        psA = stPA.enter_context(tc.tile_pool(name="psA", bufs=2, space="PSUM"))

        ident = main.tile([P, P], BF16, tag="ident")
        make_identity(nc, ident[:])

        hqT = main.tile([P, DC, TQ], BF16, tag="hqT")     # hq^T [D, TQ]
        qT = main.tile([P, DC, TQ], BF16, tag="qT")       # Q^T  [D, TQ]

        # ---- biases ----
        bqt = main.tile([P, DC], F32, tag="bqt")
        bkt = main.tile([P, DC], F32, tag="bkt")
        bot = main.tile([P, DC], F32, tag="bot")
        bft = main.tile([P, MC], F32, tag="bft")
        bvb = main.tile([P, D], F32, tag="bvb")
        nc.sync.dma_start(out=bqt[:], in_=t["bq"].rearrange("(c p) -> p c", p=P))
        nc.sync.dma_start(out=bkt[:], in_=t["bk"].rearrange("(c p) -> p c", p=P))
        nc.sync.dma_start(out=bot[:], in_=t["bo"].rearrange("(c p) -> p c", p=P))
        nc.sync.dma_start(out=bft[:], in_=t["bf_"].rearrange("(c p) -> p c", p=P))
        nc.gpsimd.dma_start(out=bvb[:], in_=_bcast_ap(t["bv"], 0, P, D))

        hT = poolAB.tile([P, DC, CTX], BF16, tag="hT")    # h^T  [D, CTX]
        vtok = poolAB.tile([P, KC, D], BF16, tag="vtok")  # V    [CTX, D]

        # ---- stage A: embedding gather + positional add + transpose ----
        with nc.named_scope("embed"), tc.tile_pool(name="ga", bufs=2) as ga:
            def embed_group(ids_dram, pe_dram, g, dstT):
                idx = ga.tile([P, 1], I32, tag="idx")
                nc.sync.dma_start(out=idx[:], in_=ids_dram[g * P:(g + 1) * P, :])
                hrow = ga.tile([P, D], F32, tag="hrow")
                nc.gpsimd.indirect_dma_start(
                    out=hrow[:], out_offset=None, in_=t["emb"][:, :],
                    in_offset=bass.IndirectOffsetOnAxis(ap=idx[:, :1], axis=0))
                pet = ga.tile([P, D], F32, tag="pet")
                nc.sync.dma_start(out=pet[:], in_=pe_dram[g * P:(g + 1) * P, :])
                hb = ga.tile([P, D], BF16, tag="hb")
                nc.vector.tensor_add(hb[:], hrow[:], pet[:])
                for dc in range(DC):
                    pt_ = psA.tile([P, P], BF16, tag="psT", space="PSUM")
                    nc.tensor.transpose(out=pt_[:], in_=hb[:, dc * P:(dc + 1) * P],
                                        identity=ident[:])
                    nc.scalar.activation(dstT[:, dc, g * P:(g + 1) * P], pt_[:],
                                         AF.Copy)

            for g in range(KC):
                embed_group(t["xk"], t["pek"], g, hT)
            for g in range(QS):
                embed_group(t["xq"], t["peq"], g, hqT)

        # ---- stage B: projections (K^T spilled to DRAM scratch) ----
        with nc.named_scope("qkv"):
            wq_s = poolW.tile([P, DC, D], BF16, tag="w_d")
            nc.sync.dma_start(out=wq_s[:], in_=t["wq_r"][:, :, :])
            for dc in range(DC):
                ps = psB.tile([P, N_TILE], F32, tag="psB", space="PSUM")
                for di in range(DC):
                    nc.tensor.matmul(out=ps[:, :TQ],
                                     lhsT=wq_s[:, di, dc * P:(dc + 1) * P],
                                     rhs=hqT[:, di, :],
                                     start=(di == 0), stop=(di == DC - 1))
                nc.scalar.activation(qT[:, dc, :], ps[:, :TQ], AF.Identity,
                                     bias=bqt[:, dc:dc + 1])

            with tc.tile_pool(name="ksp", bufs=3) as ksp:
                wk_s = poolW.tile([P, DC, D], BF16, tag="w_d")
                nc.sync.dma_start(out=wk_s[:], in_=t["wk_r"][:, :, :])
                for dc in range(DC):
                    for tc4 in range(CTX // N_TILE):
                        sl = slice(tc4 * N_TILE, (tc4 + 1) * N_TILE)
                        ps = psB.tile([P, N_TILE], F32, tag="psB", space="PSUM")
                        for di in range(DC):
                            nc.tensor.matmul(out=ps[:],
                                             lhsT=wk_s[:, di, dc * P:(dc + 1) * P],
                                             rhs=hT[:, di, sl],
                                             start=(di == 0), stop=(di == DC - 1))
                        kt_s = ksp.tile([P, N_TILE], BF16, tag="kt_s")
                        nc.scalar.activation(kt_s[:], ps[:], AF.Identity,
                                             bias=bkt[:, dc:dc + 1])
                        nc.sync.dma_start(
                            out=t["ktmp"][dc * P:(dc + 1) * P, sl], in_=kt_s[:])

                wv_s = poolW.tile([P, DC, D], BF16, tag="w_d")
                nc.sync.dma_start(out=wv_s[:], in_=t["wv_r"][:, :, :])
                for kc in range(KC):
                    for nn in range(D // N_TILE):
                        sl = slice(nn * N_TILE, (nn + 1) * N_TILE)
                        ps = psB.tile([P, N_TILE], F32, tag="psB", space="PSUM")
                        for di in range(DC):
                            nc.tensor.matmul(out=ps[:],
                                             lhsT=hT[:, di, kc * P:(kc + 1) * P],
                                             rhs=wv_s[:, di, sl],
                                             start=(di == 0), stop=(di == DC - 1))
                        nc.vector.tensor_add(vtok[:, kc, sl], ps[:], bvb[:, sl])

        # ---- stage C1: scores + softmax + transpose to P^T ----
        pT = poolC.tile([P, KC, TQ], BF16, tag="pT")      # P^T  [CTX, TQ]
        atT = poolC.tile([P, DC, TQ], BF16, tag="atT")    # attn^T
        aoT = poolC.tile([P, DC, TQ], BF16, tag="aoT")    # (attn Wo)^T

        inv_sqrt_d = 1.0 / float(np.sqrt(D))
        with nc.named_scope("attn"):
            with tc.tile_pool(name="sm", bufs=2) as sm, \
                 tc.tile_pool(name="kst", bufs=2) as kst, \
                 tc.tile_pool(name="psS", bufs=1, space="PSUM") as psS:
                for m in range(QS):
                    s_ps = psS.tile([P, CTX], F32, tag="psS", space="PSUM")
                    for kb in range(CTX // N_TILE):
                        sl = slice(kb * N_TILE, (kb + 1) * N_TILE)
                        kblk = kst.tile([P, DC, N_TILE], BF16, tag="kblk")
                        nc.sync.dma_start(out=kblk[:], in_=t["ktmp_r"][:, :, sl])
                        for di in range(DC):
                            nc.tensor.matmul(out=s_ps[:, sl],
                                             lhsT=qT[:, di, m * P:(m + 1) * P],
                                             rhs=kblk[:, di, :],
                                             start=(di == 0), stop=(di == DC - 1))
                    mx = sm.tile([P, 1], F32, tag="mx")
                    nc.vector.reduce_max(mx[:], s_ps[:], axis=AX.X)
                    negmx = sm.tile([P, 1], F32, tag="negmx")
                    nc.scalar.mul(negmx[:], mx[:], -inv_sqrt_d)
                    p0 = sm.tile([P, CTX], BF16, tag="p0")
                    nc.scalar.activation(p0[:], s_ps[:], AF.Exp,
                                         bias=negmx[:, :1], scale=inv_sqrt_d)
                    mask = sm.tile([P, CTX], BF16, tag="mask")
                    nc.sync.dma_start(out=mask[:], in_=t["maskq"][m, :, :])
                    nc.vector.tensor_mul(p0[:], p0[:], mask[:])
                    den = sm.tile([P, 1], F32, tag="den")
                    nc.vector.reduce_sum(den[:], p0[:], axis=AX.X)
                    rden = sm.tile([P, 1], F32, tag="rden")
                    nc.vector.reciprocal(rden[:], den[:])
                    nc.vector.tensor_scalar_mul(p0[:], p0[:], rden[:, :1])
                    for kc in range(KC):
                        pt_ = psA.tile([P, P], BF16, tag="psT", space="PSUM")
                        nc.tensor.transpose(out=pt_[:],
                                            in_=p0[:, kc * P:(kc + 1) * P],
                                            identity=ident[:])
                        nc.scalar.activation(pT[:, kc, m * P:(m + 1) * P], pt_[:],
                                             AF.Copy)
            stPA.close()  # transpose psum done after C1

            # ---- stage C2: attn^T = V-blocks^T @ P^T ----
            for dc in range(DC):
                ps = psB.tile([P, N_TILE], F32, tag="psB", space="PSUM")
                for kc in range(KC):
                    nc.tensor.matmul(out=ps[:, :TQ],
                                     lhsT=vtok[:, kc, dc * P:(dc + 1) * P],
                                     rhs=pT[:, kc, :],
                                     start=(kc == 0), stop=(kc == KC - 1))
                nc.scalar.activation(atT[:, dc, :], ps[:, :TQ], AF.Copy)
            stAB.close()  # hT/vtok dead after C2

            # ---- stage C3a: attnout^T = Wo^T @ attn^T ----
            wo_s = poolW.tile([P, DC, D], BF16, tag="w_d")
            nc.sync.dma_start(out=wo_s[:], in_=t["wo_r"][:, :, :])
            for dc in range(DC):
                ps = psB.tile([P, N_TILE], F32, tag="psB", space="PSUM")
                for di in range(DC):
                    nc.tensor.matmul(out=ps[:, :TQ],
                                     lhsT=wo_s[:, di, dc * P:(dc + 1) * P],
                                     rhs=atT[:, di, :],
                                     start=(di == 0), stop=(di == DC - 1))
                nc.scalar.activation(aoT[:, dc, :], ps[:, :TQ], AF.Identity,
                                     bias=bot[:, dc:dc + 1])
        stW.close()  # d->d weights dead after C3a

        # ---- stage C3b: ffT = gelu(Wf^T @ aoT + bf) ----
        ffT = poolFF.tile([P, MC, TQ], BF16, tag="ffT")   # ff^T [M, TQ]
        with nc.named_scope("ffn"), tc.tile_pool(name="poolWF", bufs=1) as poolWF:
            wf_s = poolWF.tile([P, DC, M], BF16, tag="w_f")
            nc.sync.dma_start(out=wf_s[:], in_=t["wf_r"][:, :, :])
            for mc in range(MC):
                ps = psB.tile([P, N_TILE], F32, tag="psB", space="PSUM")
                for di in range(DC):
                    nc.tensor.matmul(out=ps[:, :TQ],
                                     lhsT=wf_s[:, di, mc * P:(mc + 1) * P],
                                     rhs=aoT[:, di, :],
                                     start=(di == 0), stop=(di == DC - 1))
                nc.scalar.activation(ffT[:, mc, :], ps[:, :TQ], AF.Gelu,
                                     bias=bft[:, mc:mc + 1])

        # close remaining stage A-C pools before stage D
        stC.close()
        stPB.close()

        # ---- stage D: logits = ffT^T @ Wl + bl ----
        with nc.named_scope("logits"), \
             tc.tile_pool(name="wlp", bufs=1) as wlp, \
             tc.tile_pool(name="blp", bufs=2) as blp, \
             tc.tile_pool(name="outp", bufs=4) as outp, \
             tc.tile_pool(name="psD", bufs=4, space="PSUM") as psD:
            for (n0, nsz) in N_TILES:
                slab = wlp.tile([P, MC, N_TILE], BF16, tag="slab", bufs=2)
                for qq in range(4):
                    nc.sync.dma_start(
                        out=slab[:, qq * 8:(qq + 1) * 8, :nsz],
                        in_=t["wl_r"][:, qq * 8:(qq + 1) * 8, n0:n0 + nsz])
                blt = blp.tile([P, N_TILE], F32, tag="blt")
                nc.gpsimd.dma_start(out=blt[:, :nsz],
                                    in_=_bcast_ap(t["bl"], n0, P, nsz))
                for m in range(QS):
                    ps = psD.tile([P, N_TILE], F32, tag="psD", space="PSUM")
                    for kc in range(MC):
                        nc.tensor.matmul(out=ps[:, :nsz],
                                         lhsT=ffT[:, kc, m * P:(m + 1) * P],
                                         rhs=slab[:, kc, :nsz],
                                         start=(kc == 0), stop=(kc == MC - 1))
                    ot = outp.tile([P, N_TILE], F32, tag="ot")
                    nc.vector.tensor_add(ot[:, :nsz], ps[:, :nsz], blt[:, :nsz])
                    nc.sync.dma_start(
                        out=t["out"][m * P:(m + 1) * P, n0:n0 + nsz],
                        in_=ot[:, :nsz])


def _prep_inputs(x, emb, pe, Wq, bq, Wk, bk, Wv, bv, Wo, bo, Wf, bf, Wl, bl):
    """Host-side sharding / layout prep (no data-dependent compute)."""
    bf16 = ml_dtypes.bfloat16
    x = np.asarray(x)
    shared = {
        "emb": np.ascontiguousarray(np.asarray(emb, np.float32)),
        "pek": np.ascontiguousarray(np.asarray(pe, np.float32)[:CTX]),
        "wq": np.asarray(Wq).astype(bf16),
        "wk": np.asarray(Wk).astype(bf16),
        "wv": np.asarray(Wv).astype(bf16),
        "wo": np.asarray(Wo).astype(bf16),
        "wf": np.asarray(Wf).astype(bf16),
        "wl": np.ascontiguousarray(np.asarray(Wl).astype(bf16)),
        "bq": np.asarray(bq, np.float32),
        "bk": np.asarray(bk, np.float32),
        "bv": np.asarray(bv, np.float32),
        "bo": np.asarray(bo, np.float32),
        "bf": np.asarray(bf, np.float32),
        "bl": np.asarray(bl, np.float32),
    }
    pe32 = shared["pek"]
    in_maps = []
    for c in range(NCORES):
        b, j = divmod(c, NCORES // B)
        q0 = j * TQ
        mask = np.zeros((QS, P, CTX), dtype=bf16)
        for m in range(QS):
            gq = q0 + m * P + np.arange(P)[:, None]
            mask[m] = (np.arange(CTX)[None, :] <= gq).astype(bf16)
        im = dict(shared)
        im["xk"] = np.ascontiguousarray(x[b].astype(np.int32).reshape(CTX, 1))
        im["xq"] = np.ascontiguousarray(
            x[b, q0:q0 + TQ].astype(np.int32).reshape(TQ, 1))
        im["peq"] = np.ascontiguousarray(pe32[q0:q0 + TQ])
        im["maskq"] = mask
        in_maps.append(im)
    return in_maps


def kernel(**inputs):
    if "nc" not in _CACHE:
        _CACHE["nc"] = build_program()
    nc = _CACHE["nc"]
    in_maps = _prep_inputs(**inputs)
    res = run_bass_kernel_spmd(nc, in_maps, list(range(NCORES)))
    x = np.asarray(inputs["x"])
    Bsz, Ssz = x.shape
    out = np.empty((Bsz, Ssz, V), np.float32)
    for c in range(NCORES):
        b, j = divmod(c, NCORES // B)
        q0 = j * TQ
        out[b, q0:q0 + TQ] = res.results[c]["out"]
    return out


if __name__ == "__main__":
    pass
